# revision 1
# baseline (speedup 1.0000x reference)
"""Trainium2 Bass kernel for nn_Message_Passer (gnn_message_passing).

Reference computation:
    A = relu(edge_ij @ W + b)            # [B, E, 1024]
    messages = einsum("beij,bej->bei", A.reshape(B,E,32,32), node_j)

Strategy (8 NeuronCores, data-parallel over the flattened B*E edge dim):
  - Host pre-transposes inputs: edgeT_aug [65, BE] (64 edge features + ones row
    so the bias rides inside the matmul), nodeT [32, BE] (bf16), W_aug [65,1024].
  - matmul1 (PE, float32r single-pass mode): lhsT = W_aug column-block g,
    rhs = edgeT tile -> AT_g [128, ET] in PSUM. Partition p of bank g is
    A-column k = 128g + p, i.e. (i, j) = (k // 32, k % 32).
  - Fused relu+multiply: P = max(AT, 0) * nodeT_rep, where nodeT_rep[p, e] =
    node[e, p % 32] (a 4x-replicated [128, *] bf16 tile serves every bank).
    Done with DVE scalar_tensor_tensor straight out of PSUM; a fraction of
    bank-pairs instead goes ACT relu (PSUM->SBUF bf16) + DVE tensor_tensor at
    2x so the work splits across both engines.
  - j-reduction via PE: constant 0/1 selector matmuls (col-tiled 4x) accumulate
    sum_j P[(i,j), e] into one PSUM bank as 4 strips (rows 32c..32c+7 valid).
  - ACT copies the msg bank PSUM->SBUF [128, ET]; one DMA per tile stores the
    whole bank to msg_raw [128, E_core]; host extracts the 4 valid strips and
    transposes back to [B, E, 32] fp32.
"""

import threading

import numpy as np
import ml_dtypes

import concourse.bass as bass
import concourse.mybir as mybir
import concourse.tile as tile
from concourse import bacc
from concourse.bass import ts, ds
from concourse.bass_utils import run_bass_kernel_spmd

N_CORES = 8
B, E_FULL, ND, ED = 16, 4096, 32, 64
EDGES = B * E_FULL            # 65536
E_CORE = EDGES // N_CORES     # 8192
ET = 512                      # edges per on-chip tile
NT = E_CORE // ET             # 16 tiles
GT = 4                        # tiles per input-load group
GRP = GT * ET                 # 2048 edges per load group
KDIM = ED + 1                 # 65 (edge features + ones row for bias)
NK = ND * ND                  # 1024 A-columns
F32 = mybir.dt.float32
F32R = mybir.dt.float32r
BF16 = mybir.dt.bfloat16

# Per-tile count of PSUM bank-pairs handled by the fused DVE
# scalar_tensor_tensor path (rest: ACT-relu + DVE-tensor_tensor). Balances
# ACT (1.2 GHz, 1x) against DVE (0.96 GHz, 1x-from-PSUM / 2x-bf16).
STT_PAIRS = [1, 1, 2]  # cycled by tile index


def _build_nc(repeat: int = 1):
    nc = bacc.Bacc("TRN2", target_bir_lowering=False, debug=False,
                   num_devices=N_CORES)
    edgeT_d = nc.dram_tensor("edgeT", [KDIM, E_CORE], F32R, kind="ExternalInput")
    nodeT_d = nc.dram_tensor("nodeT", [ND, E_CORE], BF16, kind="ExternalInput")
    w_d = nc.dram_tensor("w_aug", [KDIM, NK], F32R, kind="ExternalInput")
    sel_d = nc.dram_tensor("sel", [128, 8 * ND], BF16, kind="ExternalInput")
    out_d = nc.dram_tensor("msg_raw", [128, E_CORE], F32, kind="ExternalOutput")

    with tile.TileContext(nc) as tc:
        with (
            tc.tile_pool(name="const", bufs=1) as constp,
            tc.tile_pool(name="edge", bufs=3) as edgep,
            tc.tile_pool(name="node", bufs=3) as nodep,
            tc.tile_pool(name="ar", bufs=5) as arp,
            tc.tile_pool(name="pp", bufs=8) as ppp,
            tc.tile_pool(name="mo", bufs=4) as mop,
            tc.tile_pool(name="apsum", bufs=3, space="PSUM") as apsum,
            tc.tile_pool(name="mpsum", bufs=2, space="PSUM") as mpsum,
        ):
            w_sb = constp.tile([KDIM, NK], F32R, name="w_sb")
            nc.sync.dma_start(out=w_sb[:], in_=w_d[:])
            sel_sb = constp.tile([128, 8 * ND], BF16, name="sel_sb")
            sel_loaded = False

            for t in range(NT * repeat):
                t = t % NT
                ecols = ts(t, ET)
                grp, loc = divmod(t, GT)
                if loc == 0:
                    # stream the next 4-tile group of inputs
                    gcols = ts(grp, GRP)
                    ed_sb = edgep.tile([KDIM, GRP], F32R, name="ed_sb")
                    nd_sb = nodep.tile([128, GRP], BF16, name="nd_sb")
                    if grp == 0:
                        # startup order: first edge chunk, then the node
                        # strips tile 0 needs, then the remaining chunks
                        nc.sync.dma_start(out=ed_sb[:, ts(0, ET)],
                                          in_=edgeT_d[:, ts(0, ET)])
                        for c in range(4):
                            nc.sync.dma_start(
                                out=nd_sb[32 * c:32 * (c + 1), :],
                                in_=nodeT_d[:, gcols])
                        for cc in range(1, GT):
                            nc.sync.dma_start(
                                out=ed_sb[:, ts(cc, ET)],
                                in_=edgeT_d[:, ts(cc, ET)])
                    else:
                        nc.sync.dma_start(out=ed_sb[:], in_=edgeT_d[:, gcols])
                        for c in range(4):
                            nc.sync.dma_start(
                                out=nd_sb[32 * c:32 * (c + 1), :],
                                in_=nodeT_d[:, gcols])
                lcols = ts(loc, ET)
                if not sel_loaded:
                    # sel is first needed after the first fused pair; loading
                    # it after group 0 keeps the critical DMAs in front
                    nc.sync.dma_start(out=sel_sb[:], in_=sel_d[:])
                    sel_loaded = True

                mg = mpsum.tile([128, ET], F32, name="mg")
                pend = []
                # tile 0 leans on the fused DVE path while ACT is still
                # loading its activation table
                n_stt = 2 if t == 0 else STT_PAIRS[t % len(STT_PAIRS)]
                # spread the DVE-heavy fused pairs across the tile
                stt_q = {0: (), 1: (1,), 2: (1, 3), 3: (0, 1, 3),
                         4: (0, 1, 2, 3)}[n_stt]
                for q in range(4):
                    ap_t = apsum.tile([128, 2 * ET], F32, name="ap_t")
                    for gl in range(2):
                        g = 2 * q + gl
                        # float32r: fp32 operands, single-pass (relaxed
                        # precision) PE mode — 4x faster than strict fp32
                        nc.tensor.matmul(ap_t[:, ts(gl, ET)],
                                         w_sb[:, ts(g, 128)],
                                         ed_sb[:, lcols],
                                         start=True, stop=True)
                    pp = ppp.tile([128, 2 * ET], BF16, name="pp")
                    nd_b = nd_sb[:, lcols].unsqueeze(1).broadcast_to(
                        [128, 2, ET])
                    if q in stt_q:
                        # fused relu+mult straight from PSUM on DVE
                        nc.vector.scalar_tensor_tensor(
                            out=pp[:].rearrange("p (g e) -> p g e", g=2),
                            in0=ap_t[:].rearrange("p (g e) -> p g e", g=2),
                            scalar=0.0,
                            in1=nd_b,
                            op0=mybir.AluOpType.max,
                            op1=mybir.AluOpType.mult,
                        )
                    else:
                        # relu on ACT (PSUM->SBUF bf16), multiply on DVE at 2x
                        ar = arp.tile([128, 2 * ET], BF16, name="ar")
                        nc.scalar.activation(
                            ar[:], ap_t[:], mybir.ActivationFunctionType.Relu)
                        nc.vector.tensor_tensor(
                            out=pp[:].rearrange("p (g e) -> p g e", g=2),
                            in0=ar[:].rearrange("p (g e) -> p g e", g=2),
                            in1=nd_b,
                            op=mybir.AluOpType.mult,
                        )
                    # j-reduction: strip q of the msg bank accumulates two
                    # selector matmuls (col-tiled). Lag each strip's second
                    # matmul by one pair so adjacent Sel-MMs sit in different
                    # column groups and overlap on the PE array.
                    pend.append((q, pp))
                    nc.tensor.matmul(mg[32 * q:32 * (q + 1), :],
                                     sel_sb[:, ts(2 * q, ND)],
                                     pp[:, ts(0, ET)],
                                     start=True, stop=False,
                                     skip_group_check=True,
                                     tile_position=(0, 32 * q))
                    if len(pend) > 1:
                        q0, pp0 = pend.pop(0)
                        nc.tensor.matmul(mg[32 * q0:32 * (q0 + 1), :],
                                         sel_sb[:, ts(2 * q0 + 1, ND)],
                                         pp0[:, ts(1, ET)],
                                         start=False, stop=True,
                                         skip_group_check=True,
                                         tile_position=(0, 32 * q0))

                q0, pp0 = pend.pop(0)
                nc.tensor.matmul(mg[32 * q0:32 * (q0 + 1), :],
                                 sel_sb[:, ts(2 * q0 + 1, ND)],
                                 pp0[:, ts(1, ET)],
                                 start=False, stop=True,
                                 skip_group_check=True,
                                 tile_position=(0, 32 * q0))

                mo = mop.tile([128, ET], F32, name="mo")
                if t % 3 == 2:
                    # keep ACT/DVE balanced: every third msg copy on DVE
                    nc.vector.tensor_copy(mo[:], mg[:])
                else:
                    nc.scalar.copy(mo[:], mg[:])
                nc.sync.dma_start(out=out_d[:, ecols], in_=mo[:])

    nc.compile()
    return nc


def _sel_matrix() -> np.ndarray:
    """sel[p, 32*g + m] = 1 iff m == p//32 + 4*(g%2).

    Bank g holds A-columns k = 128g + p -> i = 4g + p//32.  Strip c = g//2 of
    the msg PSUM bank accumulates banks {2c, 2c+1}; its row m carries global
    i = 8c + m, and i - 8c = p//32 + 4*(g%2)."""
    sel = np.zeros((128, 8 * ND), dtype=np.float32)
    p = np.arange(128)
    for g in range(8):
        m = p // 32 + 4 * (g % 2)
        sel[p, 32 * g + m] = 1.0
    return sel.astype(ml_dtypes.bfloat16)


_LOCK = threading.Lock()
_NC = None


def _get_nc():
    global _NC
    with _LOCK:
        if _NC is None:
            _NC = _build_nc()
    return _NC


def _prep_inputs(node_j, edge_ij, W, b):
    node_j = np.asarray(node_j, dtype=np.float32)
    edge_ij = np.asarray(edge_ij, dtype=np.float32)
    W = np.asarray(W, dtype=np.float32)
    b = np.asarray(b, dtype=np.float32)

    edge_flat = edge_ij.reshape(EDGES, ED)
    edgeT_aug = np.empty((KDIM, EDGES), dtype=np.float32)
    edgeT_aug[:ED] = edge_flat.T
    edgeT_aug[ED] = 1.0

    nodeT = np.ascontiguousarray(
        node_j.reshape(EDGES, ND).T).astype(ml_dtypes.bfloat16)

    w_aug = np.empty((KDIM, NK), dtype=np.float32)
    w_aug[:ED] = W
    w_aug[ED] = b

    sel = _sel_matrix()

    in_maps = []
    for c in range(N_CORES):
        cols = slice(c * E_CORE, (c + 1) * E_CORE)
        in_maps.append({
            "edgeT": np.ascontiguousarray(edgeT_aug[:, cols]),
            "nodeT": np.ascontiguousarray(nodeT[:, cols]),
            "w_aug": w_aug,
            "sel": sel,
        })
    return in_maps


def _extract_msgT(msg_raw: np.ndarray) -> np.ndarray:
    """[128, E_core] raw PSUM-bank image -> msgT [32, E_core]."""
    return np.concatenate([msg_raw[32 * c:32 * c + 8] for c in range(4)],
                          axis=0)


def kernel(node_j, edge_ij, W, b):
    nc = _get_nc()
    in_maps = _prep_inputs(node_j, edge_ij, W, b)
    res = run_bass_kernel_spmd(nc, in_maps, core_ids=list(range(N_CORES)))
    msgT = np.concatenate(
        [_extract_msgT(res.results[c]["msg_raw"]) for c in range(N_CORES)],
        axis=1)  # [32, EDGES]
    return np.ascontiguousarray(msgT.T).reshape(B, E_FULL, ND)

